# revision 1
# baseline (speedup 1.0000x reference)
"""Trainium2 Bass kernel for a 2-layer GAT + attention pooling (PyG-style).

Strategy (8 NeuronCores, SPMD):
  * Destination nodes are range-partitioned: core k owns dst nodes
    [k*N/8, (k+1)*N/8). Edges (self-loops appended) are sorted by dst and
    routed to the owning core (host-side index prep only).
  * conv1's dense part (h1 = x @ W1.T and per-node attention scalars) is
    computed replicated on every core into a packed DRAM table:
    row(n) = [h1(n) | a_src(n) | a_dst(n) | pad].
  * Per-edge work: dma_gather of table rows by src id plus a 256B gather
    of the attention slice by dst id. Scores p = exp(leaky(.)) stay in
    SBUF; the weighted segment-sum rides the PE via constant 0/1
    selector matmuls (edge -> local dst one-hot), accumulating numerator
    and denominator in PSUM per chunk of 128 dst nodes.
  * One AllGather exchanges conv2 table rows between the convs; one
    AllReduce combines graph-pooling partials. Final head replicated.
"""

import numpy as np
from contextlib import ExitStack

import concourse.bass as bass
import concourse.tile as tile
from concourse import bacc, mybir

F32 = mybir.dt.float32
F32R = mybir.dt.float32r
BF16 = mybir.dt.bfloat16
U8 = mybir.dt.uint8
I16 = mybir.dt.int16
AL = mybir.AluOpType
AF = mybir.ActivationFunctionType

TILE = 128
NEG = 0.2


class Cfg:
    def __init__(self, N=20000, NC=8, GRP=32, H1=8, C1=64, FIN=128, D2=128,
                 NGR=256, OUT=2):
        self.N, self.NC, self.GRP = N, NC, GRP
        self.H1, self.C1, self.FIN, self.D2 = H1, C1, FIN, D2
        self.D1 = H1 * C1
        self.NGR, self.OUT = NGR, OUT
        self.NLOC = N // NC
        self.CH = (self.NLOC + TILE - 1) // TILE
        # packed row widths (bf16 rows, multiples of 128 elems = 256B)
        self.EXT1 = -(-(self.D1 + 2 * H1) // 128) * 128
        self.EXT2 = -(-(self.D2 + 2) // 128) * 128


def build_program(cfg, tpc, t_pad, stage=99):
    c_ = cfg
    ng = t_pad // c_.GRP
    nt1 = (c_.N + TILE - 1) // TILE

    nc = bacc.Bacc("TRN2", target_bir_lowering=False, debug=False,
                   num_devices=c_.NC)

    def par(name, shape, dt=F32):
        return nc.declare_dram_parameter(name, shape, dt, isOutput=False)

    # packed parameters (param count drives per-exec binding overhead)
    smw = c_.D1 + 3 * c_.D2 + TILE + c_.OUT + 3   # smalls width
    x_p = par("x", [nt1 * TILE, c_.FIN])
    wpa = par("wpa", [c_.FIN, 2 * c_.D1])          # w1t | w2
    wpb = par("wpb", [2 * c_.D1, c_.D2])           # w1 | w2t
    apk = par("apk", [c_.D1, 2 * c_.H1 + 4])       # a1m | a2m | woutt
    gidx = par("gidx", [TILE, 2 * t_pad * 8], I16)  # gsrc | gdst
    selp = par("selp", [TILE, t_pad * TILE + c_.CH * c_.NGR], BF16)  # s01|b01
    smalls = par("smalls", [TILE, smw])
    tok = par("tok", [TILE, 8])
    w1t = wpa[:, 0:c_.D1].bitcast(F32R)
    w2 = wpa[:, c_.D1:2 * c_.D1]
    w1 = wpb[0:c_.D1, :]
    w2t = wpb[c_.D1:2 * c_.D1, :]
    a1m = apk[:, 0:2 * c_.H1]
    a2m = apk[0:c_.D2, 2 * c_.H1:2 * c_.H1 + 2]
    woutt = apk[0:c_.D2, 2 * c_.H1 + 2:2 * c_.H1 + 4]
    gsrc = gidx[:, 0:t_pad * 8]
    gdst = gidx[:, t_pad * 8:2 * t_pad * 8]
    s01 = selp[:, 0:t_pad * TILE]
    b01 = selp[:, t_pad * TILE:]
    o_ = [0]
    def sl(w):
        a = o_[0]; o_[0] += w
        return smalls[:, a:a + w]
    b1r = sl(c_.D1)
    b2r = sl(c_.D2)
    wattnr = sl(c_.D2)
    wmaskr = sl(c_.D2)
    ident = sl(TILE)
    boutr = sl(c_.OUT)
    battn = sl(1)
    epsr = sl(1)
    bmask = sl(1)

    out_p = nc.declare_dram_parameter("out", [c_.NGR, c_.OUT], F32,
                                      isOutput=True)
    tok_out = nc.declare_dram_parameter("tok_out", [TILE, 8], F32,
                                        isOutput=True)

    ext1 = nc.dram_tensor("ext1", [c_.N, c_.EXT1], BF16)
    ag_in = nc.dram_tensor("ag_in", [c_.NLOC, c_.EXT2], BF16)
    ext2 = nc.dram_tensor("ext2", [c_.N, c_.EXT2], BF16, addr_space="Shared")
    ar_in = nc.dram_tensor("ar_in", [c_.NGR, c_.D2], F32)
    pooled = nc.dram_tensor("pooled", [c_.NGR, c_.D2], F32,
                            addr_space="Shared")

    chunk_start = [c * tpc for c in range(c_.CH)]
    chunk_end = [(c + 1) * tpc for c in range(c_.CH)]
    chunk_end[c_.CH - 1] = t_pad
    chunk_of = np.zeros(t_pad, np.int32)
    for c in range(c_.CH):
        chunk_of[chunk_start[c]:chunk_end[c]] = c

    d1c = c_.D1 // TILE   # f-chunks of D1
    g_half = c_.NGR // 2

    with tile.TileContext(nc) as tc, ExitStack() as ctx:
        const = ctx.enter_context(tc.tile_pool(name="const", bufs=1))
        io = ctx.enter_context(tc.tile_pool(name="io", bufs=3))
        gat = ctx.enter_context(tc.tile_pool(name="gat", bufs=2))
        sm = ctx.enter_context(tc.tile_pool(name="sm", bufs=2))
        pp = ctx.enter_context(tc.tile_pool(name="pp", bufs=2, space="PSUM"))
        ppt = ctx.enter_context(tc.tile_pool(name="ppt", bufs=2, space="PSUM"))

        _lc = [0]
        def load_const(p, shape, dt=F32):
            _lc[0] += 1
            t = const.tile(shape, dt, tag=f"c_{_lc[0]}")
            nc.sync.dma_start(t[:], p)
            return t

        ident_t = load_const(ident, [TILE, TILE])
        tok_t = const.tile([TILE, 8], F32, tag="tok")
        nc.sync.dma_start(tok_t[:], tok[:])
        nc.sync.dma_start(tok_out[:], tok_t[:])
        w1t_t = load_const(w1t, [c_.FIN, c_.D1], F32R)
        woutt_t = load_const(woutt, [c_.D2, c_.OUT])
        b1r_t = load_const(b1r, [TILE, c_.D1])
        b2r_t = load_const(b2r, [TILE, c_.D2])
        boutr_t = load_const(boutr, [TILE, c_.OUT])
        wattn_t = load_const(wattnr, [TILE, c_.D2])
        wmask_t = load_const(wmaskr, [TILE, c_.D2])
        battn_t = load_const(battn, [TILE, 1])
        eps_t = load_const(epsr, [TILE, 1])
        bmask_t = load_const(bmask, [TILE, 1])
        gsrc_t = load_const(gsrc, [TILE, t_pad * 8], I16)
        gdst_t = load_const(gdst, [TILE, t_pad * 8], I16)
        a2m_t = load_const(a2m, [c_.D2, 2])
        w2_t = load_const(w2, [c_.D2, c_.D1])

        w2t_t = const.tile([TILE, d1c, c_.D2], F32)
        a1m_t = const.tile([TILE, d1c, 2 * c_.H1], F32)
        w1_t = const.tile([TILE, d1c, c_.FIN], F32)
        for dc in range(d1c):
            nc.sync.dma_start(w2t_t[:, dc, :], w2t[dc * TILE:(dc + 1) * TILE, :])
            nc.sync.dma_start(a1m_t[:, dc, :], a1m[dc * TILE:(dc + 1) * TILE, :])
            nc.sync.dma_start(w1_t[:, dc, :], w1[dc * TILE:(dc + 1) * TILE, :])

        # A1eff = W1.T @ A1m  [FIN, 2*H1];  A2eff = W2.T @ A2m  [D1, 2]
        a1eff_ps = ppt.tile([c_.FIN, 2 * c_.H1], F32, tag="tps")
        for dc in range(d1c):
            nc.tensor.matmul(a1eff_ps[:], w1_t[:, dc, :], a1m_t[:, dc, :],
                             start=(dc == 0), stop=(dc == d1c - 1))
        a1eff = const.tile([c_.FIN, 2 * c_.H1], F32R)
        nc.vector.tensor_copy(a1eff[:], a1eff_ps[:])

        a2eff = const.tile([TILE, d1c, 2], F32)
        for fc in range(d1c):
            a2eff_ps = ppt.tile([TILE, 2], F32, tag="small")
            nc.tensor.matmul(a2eff_ps[:], w2_t[:, fc * TILE:(fc + 1) * TILE],
                             a2m_t[:], start=True, stop=True)
            nc.vector.tensor_copy(a2eff[:, fc, :], a2eff_ps[:])

        # =========== conv1 phase 1: replicated ext1 table ===========
        XB = 4  # x tiles per DMA
        xt4 = None
        for t in range(nt1 if stage >= 1 else 0):
            if t % XB == 0:
                nb = min(XB, nt1 - t)
                xt4 = io.tile([TILE, XB, c_.FIN], F32, tag="xt")
                src = x_p[t * TILE:(t + nb) * TILE, :].rearrange(
                    "(j p) f -> p j f", p=TILE)
                nc.sync.dma_start(xt4[:, 0:nb, :], src)
            xt = xt4[:, t % XB, :]
            xT_ps = ppt.tile([TILE, TILE], F32, tag="tps")
            nc.tensor.transpose(xT_ps[:], xt, ident_t[:])
            xT = io.tile([TILE, TILE], F32R, tag="xT")
            nc.vector.tensor_copy(xT[:], xT_ps[:])
            h_ps = pp.tile([TILE, c_.D1], F32, tag="pnum")
            nc.tensor.matmul(h_ps[:], xT[:], w1t_t[:], start=True, stop=True)
            a_ps = pp.tile([TILE, 2 * c_.H1], F32, tag="pden")
            nc.tensor.matmul(a_ps[:], xT[:], a1eff[:], start=True, stop=True)
            if t % 2 == 0:
                stg = io.tile([TILE, 2, c_.EXT1], BF16, tag="stg")
            sv = stg[:, t % 2, :]
            nc.scalar.copy(sv[:, 0:c_.D1], h_ps[:])
            nc.scalar.copy(sv[:, c_.D1:c_.D1 + 2 * c_.H1], a_ps[:])
            nc.scalar.memzero(sv[:, c_.D1 + 2 * c_.H1:c_.EXT1])
            if t % 2 == 1 or t == nt1 - 1:
                t0w = t - (t % 2)
                rows = min(2 * TILE, c_.N - t0w * TILE)
                dst = ext1[t0w * TILE:t0w * TILE + rows, :].rearrange(
                    "(j p) e -> p j e", p=TILE) if rows == 2 * TILE else \
                    ext1[t0w * TILE:t0w * TILE + rows, :].unsqueeze(1)
                nj = 2 if rows == 2 * TILE else 1
                nc.sync.dma_start(dst, stg[0:min(rows, TILE), 0:nj, :])

        h_own = const.tile([TILE, c_.CH, c_.D1], F32)

        # =========== shared edge-aggregation pipeline ===========
        def conv_phase2(ext_tab, ecols, acol, nh, dfeat, bias_t, dst_sb):
            cph = dfeat // nh
            psn = psd = None
            for g in range(ng):
                extg = gat.tile([TILE, c_.GRP, ecols], BF16, tag="extg")
                nc.gpsimd.dma_gather(
                    extg[:], ext_tab[:, 0:ecols],
                    gsrc_t[:, g * c_.GRP * 8:(g + 1) * c_.GRP * 8],
                    c_.GRP * TILE, c_.GRP * TILE, ecols, elem_step=ecols,
                    single_packet=False)
                adg = gat.tile([TILE, c_.GRP, 128], BF16, tag="adg")
                nc.gpsimd.dma_gather(
                    adg[:], ext_tab[:, acol:acol + 128],
                    gdst_t[:, g * c_.GRP * 8:(g + 1) * c_.GRP * 8],
                    c_.GRP * TILE, c_.GRP * TILE, 128, elem_step=ecols,
                    single_packet=False)
                s01g = gat.tile([TILE, c_.GRP * TILE], BF16, tag="s01g")
                nc.sync.dma_start(
                    s01g[:], s01[:, g * c_.GRP * TILE:(g + 1) * c_.GRP * TILE])

                sa = extg[:, :, acol:acol + nh]
                nc.vector.tensor_tensor(sa, sa, adg[:, :, nh:2 * nh], AL.add)
                tmp = sm.tile([TILE, c_.GRP, nh], BF16, tag="tmp")
                nc.scalar.mul(tmp[:], sa, NEG)
                nc.vector.tensor_tensor(sa, sa, tmp[:], AL.max)
                nc.scalar.activation(sa, sa, AF.Exp)
                ev = extg[:, :, 0:dfeat].rearrange("p g (h c) -> p g h c", h=nh)
                pb = extg[:, :, acol:acol + nh].unsqueeze(3).broadcast_to(
                    [TILE, c_.GRP, nh, cph])
                nc.vector.tensor_tensor(ev, ev, pb, AL.mult)

                for tl in range(c_.GRP):
                    t = g * c_.GRP + tl
                    c = int(chunk_of[t])
                    first = t == chunk_start[c]
                    last = t == chunk_end[c] - 1
                    if first:
                        psn = pp.tile([TILE, dfeat], F32, tag="pnum")
                        psd = pp.tile([TILE, 2 * c_.H1], F32, tag="pden")
                    lhs = s01g[:, tl * TILE:(tl + 1) * TILE]
                    nc.tensor.matmul(psn[:], lhs, extg[:, tl, 0:dfeat],
                                     start=first, stop=last)
                    nc.tensor.matmul(psd[:, 0:nh], lhs,
                                     extg[:, tl, acol:acol + nh],
                                     start=first, stop=last)
                    if last:
                        den = sm.tile([TILE, nh], F32, tag="den")
                        nc.scalar.activation(den[:], psd[:, 0:nh], AF.Identity,
                                             bias=eps_t[:])
                        denr = sm.tile([TILE, nh], F32, tag="denr")
                        nc.vector.reciprocal(denr[:], den[:])
                        ov = dst_sb[:, c, :].rearrange("p (h c) -> p h c", h=nh)
                        nv = psn[:].rearrange("p (h c) -> p h c", h=nh)
                        db = denr[:].unsqueeze(2).broadcast_to([TILE, nh, cph])
                        nc.vector.tensor_tensor(ov, nv, db, AL.mult)
                        nc.vector.tensor_tensor(dst_sb[:, c, :],
                                                dst_sb[:, c, :], bias_t[:],
                                                AL.add)
                        nc.scalar.activation(dst_sb[:, c, :], dst_sb[:, c, :],
                                             AF.Relu)

        if stage >= 2:
            conv_phase2(ext1, c_.EXT1, c_.D1, c_.H1, c_.D1, b1r_t, h_own)
        else:
            nc.vector.memset(h_own[:], 0.0)

        # =========== conv2 phase 1: own h2 rows -> AllGather ===========
        for c in range(c_.CH if stage >= 3 else 0):
            h2_ps = pp.tile([TILE, c_.D2], F32, tag="pnum")
            a2_ps = pp.tile([TILE, 2], F32, tag="pden")
            for fc in range(d1c):
                hT_ps = ppt.tile([TILE, TILE], F32, tag="tps")
                nc.tensor.transpose(
                    hT_ps[:], h_own[:, c, fc * TILE:(fc + 1) * TILE], ident_t[:])
                hT = io.tile([TILE, TILE], F32, tag="xT")
                nc.vector.tensor_copy(hT[:], hT_ps[:])
                nc.tensor.matmul(h2_ps[:], hT[:], w2t_t[:, fc, :],
                                 start=(fc == 0), stop=(fc == d1c - 1))
                nc.tensor.matmul(a2_ps[:], hT[:], a2eff[:, fc, :],
                                 start=(fc == 0), stop=(fc == d1c - 1))
            stg2 = io.tile([TILE, c_.EXT2], BF16, tag="stg")
            nc.scalar.copy(stg2[:, 0:c_.D2], h2_ps[:])
            nc.scalar.copy(stg2[:, c_.D2:c_.D2 + 2], a2_ps[:])
            nc.scalar.memzero(stg2[:, c_.D2 + 2:c_.EXT2])
            rows = min(TILE, c_.NLOC - c * TILE)
            nc.sync.dma_start(ag_in[c * TILE:c * TILE + rows, :],
                              stg2[0:rows, :])

        if stage >= 3:
            nc.gpsimd.collective_compute(
                "AllGather", AL.bypass, replica_groups=[list(range(c_.NC))],
                ins=[ag_in[:]], outs=[ext2[:]])

        # =========== conv2 phase 2 ===========
        o2_own = const.tile([TILE, c_.CH, c_.D2], F32)
        if stage >= 4:
            conv_phase2(ext2, c_.EXT2, c_.D2, 1, c_.D2, b2r_t, o2_own)
        else:
            nc.vector.memset(o2_own[:], 0.0)

        # =========== pooling partials + AllReduce ===========
        ps_pa = pp.tile([TILE, c_.D2], F32, tag="pnum")
        ps_pb = pp.tile([TILE, c_.D2], F32, tag="pden")
        for c in range(c_.CH if stage >= 5 else 1):
            h = o2_own[:, c, :]
            ta = sm.tile([TILE, c_.D2], F32, tag="ta")
            nc.vector.tensor_tensor(ta[:], h, wattn_t[:], AL.mult)
            sa = sm.tile([TILE, 1], F32, tag="sa")
            nc.vector.tensor_reduce(sa[:], ta[:], mybir.AxisListType.X, AL.add)
            nc.scalar.activation(sa[:], sa[:], AF.Identity, bias=battn_t[:])
            tm = sm.tile([TILE, c_.D2], F32, tag="ta")
            nc.vector.tensor_tensor(tm[:], h, wmask_t[:], AL.mult)
            sb = sm.tile([TILE, 1], F32, tag="sb")
            nc.vector.tensor_reduce(sb[:], tm[:], mybir.AxisListType.X, AL.add)
            nc.scalar.activation(sb[:], sb[:], AF.Sigmoid, bias=bmask_t[:])
            fac = sm.tile([TILE, 1], F32, tag="fac")
            nc.vector.tensor_tensor(fac[:], sa[:], sb[:], AL.mult)
            wn = sm.tile([TILE, c_.D2], BF16, tag="wn")
            nc.scalar.activation(wn[:], h, AF.Copy, scale=fac[:])
            b01g = sm.tile([TILE, c_.NGR], BF16, tag="b01g")
            nc.sync.dma_start(b01g[:], b01[:, c * c_.NGR:(c + 1) * c_.NGR])
            nc.tensor.matmul(ps_pa[:], b01g[:, 0:g_half], wn[:],
                             start=(c == 0), stop=(c == c_.CH - 1))
            nc.tensor.matmul(ps_pb[:], b01g[:, g_half:c_.NGR], wn[:],
                             start=(c == 0), stop=(c == c_.CH - 1))
        for half, ps in ((0, ps_pa), (1, ps_pb)):
            pl = io.tile([g_half, c_.D2], F32, tag="pl")
            nc.vector.tensor_copy(pl[:], ps[0:g_half, :])
            nc.sync.dma_start(ar_in[half * g_half:(half + 1) * g_half, :],
                              pl[:])

        if stage >= 1:
            nc.gpsimd.collective_compute(
                "AllReduce", AL.add, replica_groups=[list(range(c_.NC))],
                ins=[ar_in[:]], outs=[pooled[:]])
        else:
            nc.sync.dma_start(pooled[:], ar_in[:])

        # =========== final head (replicated) ===========
        for half in range(2):
            pf = io.tile([g_half, c_.D2], F32, tag="pl")
            nc.sync.dma_start(pf[:],
                              pooled[half * g_half:(half + 1) * g_half, :])
            pT_ps = ppt.tile([c_.D2, g_half], F32, tag="tps")
            nc.tensor.transpose(pT_ps[:], pf[:], ident_t[0:g_half, 0:g_half])
            pT = io.tile([c_.D2, g_half], F32, tag="xT")
            nc.vector.tensor_copy(pT[:], pT_ps[:])
            o_ps = ppt.tile([g_half, c_.OUT], F32, tag="small")
            nc.tensor.matmul(o_ps[:], pT[:], woutt_t[:], start=True, stop=True)
            ot = io.tile([g_half, c_.OUT], F32, tag="ot")
            nc.vector.tensor_tensor(ot[:], o_ps[:], boutr_t[0:g_half, :],
                                    AL.add)
            nc.sync.dma_start(out_p[half * g_half:(half + 1) * g_half, :],
                              ot[:])

    nc.compile()
    return nc


def host_prep(inputs, cfg):
    c_ = cfg
    ei = np.asarray(inputs["edge_index"], np.int64)
    batch = np.asarray(inputs["batch"], np.int64)

    loops = np.arange(c_.N, dtype=np.int64)
    src = np.concatenate([ei[0], loops])
    dst = np.concatenate([ei[1], loops])
    order = np.argsort(dst, kind="stable")
    src_s = src[order]
    dst_s = dst[order]

    tpc = 0
    lo_hi = []
    for k in range(c_.NC):
        row = []
        for c in range(c_.CH):
            d0 = k * c_.NLOC + c * TILE
            d1 = min(k * c_.NLOC + c_.NLOC, d0 + TILE)
            lo = int(np.searchsorted(dst_s, d0))
            hi = int(np.searchsorted(dst_s, d1))
            row.append((lo, hi, d0))
            tpc = max(tpc, -(-(hi - lo) // TILE))
        lo_hi.append(row)
    t_pad = -(-(c_.CH * tpc) // c_.GRP) * c_.GRP

    def wrap_idx(a):
        w = a.reshape(-1, 16).T.astype(np.int16)
        return np.tile(w, (8, 1)).copy()

    per_core = []
    for k in range(c_.NC):
        gi_src = np.zeros(t_pad * TILE, np.int64)
        gi_dst = np.zeros(t_pad * TILE, np.int64)
        import ml_dtypes
        s01 = np.zeros((TILE, t_pad * TILE), ml_dtypes.bfloat16)
        for c in range(c_.CH):
            lo, hi, d0 = lo_hi[k][c]
            cnt = hi - lo
            j = c * tpc * TILE + np.arange(cnt)
            gi_src[j] = src_s[lo:hi]
            gi_dst[j] = dst_s[lo:hi]
            s01[j % TILE, (j // TILE) * TILE + (dst_s[lo:hi] - d0)] = 1
        b01 = np.zeros((TILE, c_.CH * c_.NGR), ml_dtypes.bfloat16)
        ii = np.arange(c_.NLOC)
        b01[ii % TILE, (ii // TILE) * c_.NGR + batch[k * c_.NLOC + ii]] = 1
        per_core.append({"gsrc": wrap_idx(gi_src), "gdst": wrap_idx(gi_dst),
                         "s01": s01, "b01": b01})
    return tpc, t_pad, per_core


def make_in_maps(inputs, cfg, per_core):
    c_ = cfg
    x = np.asarray(inputs["x"], np.float32)
    nt1 = (c_.N + TILE - 1) // TILE
    x_pad = np.zeros((nt1 * TILE, c_.FIN), np.float32)
    x_pad[:c_.N] = x

    W1 = np.asarray(inputs["W1"], np.float32)
    as1 = np.asarray(inputs["att_src1"], np.float32)
    ad1 = np.asarray(inputs["att_dst1"], np.float32)
    W2 = np.asarray(inputs["W2"], np.float32)
    as2 = np.asarray(inputs["att_src2"], np.float32)
    ad2 = np.asarray(inputs["att_dst2"], np.float32)
    a1m = np.zeros((c_.D1, 2 * c_.H1), np.float32)
    for h in range(c_.H1):
        a1m[h * c_.C1:(h + 1) * c_.C1, h] = as1[h]
        a1m[h * c_.C1:(h + 1) * c_.C1, c_.H1 + h] = ad1[h]
    a2m = np.stack([as2[0], ad2[0]], axis=1).astype(np.float32)

    rep = lambda v, w: np.tile(
        np.asarray(v, np.float32).reshape(1, w), (TILE, 1))
    wpa = np.concatenate([np.ascontiguousarray(W1.T), W2], axis=1)
    wpb = np.concatenate([W1, np.ascontiguousarray(W2.T)], axis=0)
    apk = np.zeros((c_.D1, 2 * c_.H1 + 4), np.float32)
    apk[:, 0:2 * c_.H1] = a1m
    apk[0:c_.D2, 2 * c_.H1:2 * c_.H1 + 2] = a2m
    apk[0:c_.D2, 2 * c_.H1 + 2:2 * c_.H1 + 4] = np.ascontiguousarray(
        np.asarray(inputs["W_out"], np.float32).T)
    smalls = np.concatenate([
        rep(inputs["b1"], c_.D1), rep(inputs["b2"], c_.D2),
        rep(np.asarray(inputs["w_attn"], np.float32)[:, 0], c_.D2),
        rep(np.asarray(inputs["w_mask"], np.float32)[:, 0], c_.D2),
        np.eye(TILE, dtype=np.float32),
        rep(inputs["b_out"], c_.OUT),
        rep(inputs["b_attn"], 1),
        np.full((TILE, 1), 1e-16, np.float32),
        rep(inputs["b_mask"], 1),
    ], axis=1)
    base = {
        "x": x_pad, "wpa": wpa, "wpb": wpb, "apk": apk, "smalls": smalls,
        "tok": np.zeros((TILE, 8), np.float32),
    }
    in_maps = []
    for k in range(c_.NC):
        m = dict(base)
        pc = per_core[k]
        m["gidx"] = np.concatenate([pc["gsrc"], pc["gdst"]], axis=1)
        m["selp"] = np.concatenate([pc["s01"], pc["b01"]], axis=1)
        in_maps.append(m)
    return in_maps


_CACHE = {}


def run(inputs, cfg):
    from concourse.bass_utils import run_bass_kernel_spmd
    tpc, t_pad, per_core = host_prep(inputs, cfg)
    key = (cfg.N, tpc, t_pad)
    if key not in _CACHE:
        _CACHE[key] = build_program(cfg, tpc, t_pad)
    nc = _CACHE[key]
    in_maps = make_in_maps(inputs, cfg, per_core)
    res = run_bass_kernel_spmd(nc, in_maps, list(range(cfg.NC)), trace=False)
    return np.asarray(res.results[0]["out"], np.float32)


def kernel(**inputs):
    return run(inputs, Cfg())


def _exec_maker(nc, in_maps, n_cores):
    """Build a jitted executor (structure identical to bass2jax's _body) and
    device-resident inputs. Returns (f, dev_args)."""
    import jax
    from jax.sharding import Mesh, PartitionSpec, NamedSharding
    from jax.experimental.shard_map import shard_map
    from concourse import mybir as mb
    from concourse.bass2jax import _bass_exec_p, partition_id_tensor, \
        install_neuronx_cc_hook

    install_neuronx_cc_hook()
    partition_name = (nc.partition_id_tensor.name
                      if nc.partition_id_tensor else None)
    in_names, out_names, out_avals, zero_outs = [], [], [], []
    for alloc in nc.m.functions[0].allocations:
        if not isinstance(alloc, mb.MemoryLocationSet):
            continue
        name = alloc.memorylocations[0].name
        if alloc.kind == "ExternalInput":
            if name != partition_name:
                in_names.append(name)
        elif alloc.kind == "ExternalOutput":
            out_names.append(name)
            shape = tuple(alloc.tensor_shape)
            dtype = mb.dt.np(alloc.dtype)
            out_avals.append(jax.core.ShapedArray(shape, dtype))
            zero_outs.append(np.zeros(shape, dtype))
    n_params = len(in_names)
    all_in = in_names + out_names
    if partition_name is not None:
        all_in = all_in + [partition_name]

    def _body(*args):
        ops = list(args)
        if partition_name is not None:
            ops.append(partition_id_tensor())
        outs = _bass_exec_p.bind(
            *ops, out_avals=tuple(out_avals), in_names=tuple(all_in),
            out_names=tuple(out_names), lowering_input_output_aliases=(),
            sim_require_finite=True, sim_require_nnan=True, nc=nc)
        return tuple(outs)

    devices = jax.devices()[:n_cores]
    mesh = Mesh(np.asarray(devices), ("core",))
    nin = n_params + len(zero_outs)
    f = jax.jit(shard_map(
        _body, mesh=mesh, in_specs=(PartitionSpec("core"),) * nin,
        out_specs=(PartitionSpec("core"),) * len(out_names),
        check_rep=False), keep_unused=True)
    per_core = [[np.asarray(in_maps[c][n]) for n in in_names] + zero_outs
                for c in range(n_cores)]
    concat_in = [np.concatenate([per_core[c][i] for c in range(n_cores)],
                                axis=0) for i in range(nin)]
    sh = NamedSharding(mesh, PartitionSpec("core"))
    dev_args = [jax.device_put(a, sh) for a in concat_in]
    return f, dev_args


def _build_tiny(n_cores):
    nc = bacc.Bacc("TRN2", target_bir_lowering=False, debug=False,
                   num_devices=n_cores)
    tok = nc.declare_dram_parameter("tok", [TILE, 8], F32, isOutput=False)
    tok_out = nc.declare_dram_parameter("tok_out", [TILE, 8], F32,
                                        isOutput=True)
    with tile.TileContext(nc) as tc, ExitStack() as ctx:
        pool = ctx.enter_context(tc.tile_pool(name="p", bufs=1))
        t = pool.tile([TILE, 8], F32)
        nc.sync.dma_start(t[:], tok[:])
        nc.sync.dma_start(tok_out[:], t[:])
    nc.compile()
    return nc


def _timed_pair(f, dev_args, tf, tdev, reps):
    """Interleave kernel and baseline calls; return per-pair differences."""
    import jax
    import time as _t
    jax.block_until_ready(f(*dev_args))
    jax.block_until_ready(tf(*tdev))
    diffs = []
    for _ in range(reps):
        t0 = _t.perf_counter()
        jax.block_until_ready(f(*dev_args))
        t1 = _t.perf_counter()
        jax.block_until_ready(tf(*tdev))
        t2 = _t.perf_counter()
        diffs.append((t1 - t0) - (t2 - t1))
    diffs.sort()
    return diffs


def measure_hw_time(inputs, reps=30, cfg=None, stage=99):
    """Per-execution device time estimate: wall time of the kernel with
    device-resident inputs, minus the same measurement for a trivial
    pass-through program (dispatch/tunnel baseline)."""
    cfg = cfg or Cfg()
    tpc, t_pad, per_core = host_prep(inputs, cfg)
    key = (cfg.N, tpc, t_pad, stage)
    if key not in _CACHE:
        _CACHE[key] = build_program(cfg, tpc, t_pad, stage=stage)
    nc = _CACHE[key]
    in_maps = make_in_maps(inputs, cfg, per_core)
    f, dev_args = _exec_maker(nc, in_maps, cfg.NC)
    tiny = _build_tiny(cfg.NC)
    tiny_maps = [{"tok": np.zeros((TILE, 8), np.float32)}
                 for _ in range(cfg.NC)]
    tf, tdev = _exec_maker(tiny, tiny_maps, cfg.NC)
    diffs = _timed_pair(f, dev_args, tf, tdev, reps)
    med = diffs[len(diffs) // 2]
    lo = diffs[len(diffs) // 4]
    hi = diffs[3 * len(diffs) // 4]
    print(f"paired diff: p25={lo*1e3:.2f} med={med*1e3:.2f} "
          f"p75={hi*1e3:.2f} ms (n={reps})")
    return med * 1e9

